# revision 16
# baseline (speedup 1.0000x reference)
"""Trainium2 Bass kernel for nn_ARGCNNet (2-layer gated relational GCN), v3.

Strategy (8 NeuronCores, graph/data parallel):
  - Nodes sharded by row: core c owns nodes [c*6250, (c+1)*6250).
  - Edges routed to the core owning their dst node, sorted by (dst window,
    slab part), packed into 128-edge chunks (padding uniform across cores
    -> one SPMD program).
  - Message table is DOUBLE-FP8 (hi,lo): xt row = [fp8(xt) | fp8(xt-hi)]
    -> 512B rows, ~bf16 accuracy, and fp8 DoubleRow matmuls (2x PE rate)
    with host-duplicated alpha one-hot columns.
  - AllGathers split into 3 node slabs (fire as dense1/edge1 complete each
    slab); edges split into 3 parts by the slab holding their (permuted)
    src row so gathers chase the AllGathers.
  - Per-edge src rows fetched with few large dma_gather calls (multi-packet)
    round-robined over all 4 SWDGE queues.
  - Layer-2 aggregation: psum[dst,16] via lhsT=OH2 (bf16 alpha one-hot),
    rhs = gathered ht rows [slot,16] bf16.
  - Dropout masks host-precomputed 0/1 fp8; 1/(1-p) folded into ReLU scale.
"""

import os
import sys

import numpy as np

for _p in ("/opt/trn_rl_repo", "/root/.axon_site/_ro/trn_rl_repo"):
    if os.path.isdir(_p) and _p not in sys.path:
        sys.path.insert(0, _p)

import ml_dtypes

bf16 = ml_dtypes.bfloat16
f8 = ml_dtypes.float8_e4m3

N_NODES = 50000
N_EDGES = 800000
IN_DIM = 768
HID = 256
OUT = 9
OUTP = 16
N_TYPES = 50
N_DIST = 128
P_DROP = np.float32(0.4)
INV_KEEP = float(np.float32(1.0) / (np.float32(1.0) - P_DROP))

NCORES = 8
SHARD = N_NODES // NCORES  # 6250
P = 128
NW = (SHARD + P - 1) // P  # 49 windows per core
PADN = NW * P  # 6272
KT1 = IN_DIM // P  # 6
KT2 = HID // P  # 2
DDS = 24576
GW = 2  # windows per group (layer 1)
GW2 = 4  # windows per group (layer 2)

# node slabs (core-local row ranges) for the chunked AllGathers.
SLAB_STARTS = [0, 2048, 4096]
SLAB_LENS = [2048, 2048, SHARD - 4096]  # 2048, 2048, 2154
SLAB_WEND = [16, 32, NW]  # dense window idx (exclusive) per slab
NSLAB = 3
# permuted-table boundaries: slab s spans [8*start, 8*(start+len))
PART_BOUNDS = [0, 16384, 32768, 8 * (4096 + SLAB_LENS[2])]

IDXCAP = 1 << 30  # max indices per gather call (tuned by microbench)
SINGLE_PACKET = False


def _perm_ids():
    """Global node id -> permuted table row id (slab-concatenated AllGather
    layout: table = [slab0: core0..7 | slab1: core0..7 | slab2: ...])."""
    ids = np.arange(N_NODES, dtype=np.int64)
    c = ids // SHARD
    r = ids % SHARD
    s = np.zeros_like(r)
    for k in range(1, NSLAB):
        s += r >= SLAB_STARTS[k]
    starts = np.asarray(SLAB_STARTS, dtype=np.int64)[s]
    lens = np.asarray(SLAB_LENS, dtype=np.int64)[s]
    return 8 * starts + c * lens + (r - starts)


def _wrap_idx(flat):
    """int16 flat index list -> [128, n/16] wrapped + replicated layout."""
    n = flat.size
    assert n % 16 == 0
    t = np.empty((P, n // 16), np.int16)
    for p in range(16):
        row = flat[p::16]
        for g in range(8):
            t[16 * g + p, :] = row
    return t


def _edge_alphas(et, ed, te, de, gw, gb):
    tg = te.astype(np.float64) @ gw[:100, 0].astype(np.float64)
    dg = de.astype(np.float64) @ gw[100:, 0].astype(np.float64)
    z = tg[et] + dg[ed] + float(gb)
    return (1.0 / (1.0 + np.exp(-z))).astype(np.float32)


def _layout(src_key, part_of, dst, owner, alph, npart, gw, bounds):
    """Generic edge packing: per (group, part) contiguous columns.

    Returns meta {cw, col, calls, groups, C} and per-core (src16, OH).
    """
    per_core = []
    cnt = np.zeros((NCORES, npart, NW), np.int64)
    for c in range(NCORES):
        m = owner == c
        dstl = dst[m] - c * SHARD
        ps_ = src_key[m]
        pt = part_of[m]
        wid = dstl >> 7
        # sort by (window, part, src) so gather reads are near-sequential
        order = np.lexsort((ps_, pt, wid))
        per_core.append((dstl[order], ps_[order], alph[m][order], pt[order]))
        for p in range(npart):
            cnt[c, p] = np.bincount(wid[pt == p], minlength=NW)

    cw = np.maximum(1, (cnt.max(axis=0) + P - 1) // P)  # [npart, NW]

    groups = [list(range(g, min(g + gw, NW))) for g in range(0, NW, gw)]
    col = {}
    calls = []  # per group: list of (part, col0, ncols)
    cur = 0
    for ws in groups:
        gc = []
        for p in range(npart):
            c0 = cur
            for w in ws:
                col[(p, w)] = cur
                cur += int(cw[p, w])
            gc.append((p, c0, cur - c0))
        calls.append(gc)
    C = cur

    meta = {"cw": cw, "col": col, "calls": calls, "groups": groups, "C": C}

    col_arr = np.zeros((npart, NW), np.int64)
    for p in range(npart):
        for w in range(NW):
            col_arr[p, w] = col[(p, w)]

    out = []
    for c in range(NCORES):
        dstl, ps_, ac, pt = per_core[c]
        wid = dstl >> 7
        keys = wid * npart + pt
        cntc = np.bincount(keys, minlength=npart * NW)
        start = np.concatenate([[0], np.cumsum(cntc)[:-1]])
        rank = np.arange(dstl.size) - start[keys]
        colbase = col_arr[pt, wid]
        slot = (colbase + (rank >> 7)) * P + (rank & 127)

        rel = ps_ - np.asarray(bounds, np.int64)[pt]
        srcrel = np.zeros(C * P, np.int16)
        srcrel[slot] = rel.astype(np.int16)

        # compact one-hot descriptors: per slot the dst-low (or -1 for pad)
        # and alpha; the device builds the one-hot via DVE is_equal.
        dl = np.full(C * P, -1.0, np.float32)
        dl[slot] = (dstl & 127).astype(np.float32)
        al = np.zeros(C * P, np.float32)
        al[slot] = ac
        dl = dl.reshape(C, P).T.copy()  # [p, col]
        al = al.reshape(C, P).T.copy()
        out.append((_wrap_idx(srcrel), dl.astype(bf16), al.astype(bf16)))
    return meta, out


def _prep_edges(edge_index, a1, a2):
    """Route/sort/pack edges for both layers.

    Layer 1: 3 parts by AllGather slab of the permuted xt table (pipelines
    gathers behind the chunked AG). Layer 2: 2 parts by plain global src id
    (int16 gather index limit); the compact ht table is core-major.
    """
    src = np.asarray(edge_index[0]).astype(np.int64)
    dst = np.asarray(edge_index[1]).astype(np.int64)
    owner = dst // SHARD
    perm = _perm_ids()
    psrc = perm[src]

    part1 = np.zeros(N_EDGES, np.int64)
    for k in range(1, NSLAB):
        part1 += psrc >= PART_BOUNDS[k]
    meta1, pc1 = _layout(psrc, part1, dst, owner, a1, NSLAB, GW,
                         PART_BOUNDS)

    # layer 2: table row id in the p-major compact ht layout:
    # row = core*PADN + (n%128)*NW + n//128
    n_ = src % SHARD
    l2src = (src // SHARD) * PADN + (n_ % P) * NW + n_ // P
    part2 = (l2src >= 32768).astype(np.int64)
    meta2, pc2 = _layout(l2src, part2, dst, owner, a2, 2, GW2,
                         [0, 32768, 8 * PADN])

    meta = {"l1": meta1, "l2": meta2}
    per_core_arrays = []
    for c in range(NCORES):
        s16a, dl1, al1 = pc1[c]
        s16b, dl2, al2 = pc2[c]
        per_core_arrays.append((s16a, dl1, al1, s16b, dl2, al2))
    return meta, per_core_arrays


def _build_program(meta, sim_mode=False):
    import concourse.bacc as bacc
    import concourse.bass as bass  # noqa: F401
    import concourse.mybir as mybir
    import concourse.tile as tile

    A = mybir.AluOpType
    F = mybir.ActivationFunctionType
    dt = mybir.dt
    DR = mybir.MatmulPerfMode.DoubleRow

    m1_ = meta["l1"]
    m2_ = meta["l2"]
    C = m1_["C"]
    col = m1_["col"]
    cw = m1_["cw"]
    calls = m1_["calls"]
    groups = m1_["groups"]
    C2 = m2_["C"]
    col2 = m2_["col"]
    cw2 = m2_["cw"]
    calls2 = m2_["calls"]
    groups2 = m2_["groups"]

    nc = bacc.Bacc(
        "TRN2", target_bir_lowering=False, debug=False,
        num_devices=(1 if sim_mode else NCORES),
        dynamic_dma_scratch_size=DDS,
        num_swdge_queues=4,
    )

    def inp(name, shape, d):
        return nc.dram_tensor(name, shape, d, kind="ExternalInput")

    xTw = inp("xTw", [NW, P, IN_DIM], dt.bfloat16)
    W1 = inp("W1", [IN_DIM, 2 * HID], dt.bfloat16)  # [msg | root]
    W2 = inp("W2", [HID, 2 * OUTP], dt.bfloat16)  # [msg | root] padded
    b1row = inp("b1row", [1, HID], dt.bfloat16)
    b2row = inp("b2row", [1, OUTP], dt.bfloat16)
    ones_bf = inp("ones_bf", [1, P], dt.bfloat16)
    ident_in = inp("ident", [P, P], dt.bfloat16)
    m1_in = inp("m1w", [P, NW, HID], dt.float8e4)
    m2_in = inp("m2w", [P, NW, OUTP], dt.float8e4)
    src16_in = inp("src16", [P, C * 8], dt.int16)
    src16b_in = inp("src16b", [P, C2 * 8], dt.int16)
    dl1_in = inp("dl1", [P, C], dt.bfloat16)
    al1_in = inp("al1", [P, C], dt.bfloat16)
    dl2_in = inp("dl2", [P, C2], dt.bfloat16)
    al2_in = inp("al2", [P, C2], dt.bfloat16)
    iota4_in = inp("iota4", [P, 4 * P], dt.bfloat16)

    yL = nc.dram_tensor("yL", [P, NW * OUTP], dt.float32,
                        kind="ExternalOutput")

    xt_loc = nc.dram_tensor("xt_loc", [PADN, 2 * HID], dt.float8e4,
                            kind="Internal")
    xt_full = nc.dram_tensor(
        "xt_full", [N_NODES, 2 * HID], dt.float8e4, kind="Internal",
        addr_space="Shared",
    )
    htc_loc = nc.dram_tensor("htc_loc", [P, NW * OUTP], dt.bfloat16,
                             kind="Internal")
    htc_full = nc.dram_tensor(
        "htc_full", [8 * PADN + 8192, OUTP], dt.bfloat16, kind="Internal",
        addr_space="Shared",
    )
    htp = nc.dram_tensor("htp", [8 * PADN + 8192, P], dt.bfloat16,
                         kind="Internal")

    rg = [list(range(NCORES))]
    _qrr = [0]

    def dg_raw(out_ap, in_ap, idxs_ap, num_idxs, elem_size, stride_256):
        eng = nc.gpsimd
        q = _qrr[0]
        _qrr[0] = (q + 1) % 4
        _in_ap = eng.lower_ap_dma(in_ap, for_custom_bir_dma=True)
        _idxs_ap = eng.lower_ap(idxs_ap)
        _out_ap = eng.lower_ap(out_ap)
        return eng.add_instruction(
            mybir.InstDMAGatherAnt(
                name=nc.get_next_instruction_name(),
                ins=[*_in_ap, _idxs_ap,
                     eng.lower_val_access(eng.to_reg(num_idxs))],
                outs=[_out_ap],
                transpose=False,
                num_idxs=num_idxs,
                elem_size=elem_size,
                stride_bytes_256=stride_256,
                gen_mode=0,
                single_packet=SINGLE_PACKET,
                queue_num=q,
                sbuf_tokens_per_rank=0,
                sbuf_free_dim_per_rank=0,
                sbuf_free_dim_pad_per_rank=0,
                sbuf_byte_offset=0,
            )
        )

    def allgather(src_dram, dst_dram, s):
        a, ln = SLAB_STARTS[s], SLAB_LENS[s]
        if sim_mode:
            for cc in range(NCORES):
                nc.sync.dma_start(
                    dst_dram[8 * a + cc * ln: 8 * a + (cc + 1) * ln, :],
                    src_dram[a: a + ln, :],
                )
        else:
            nc.gpsimd.collective_compute(
                "AllGather",
                A.bypass,
                replica_groups=rg,
                ins=[src_dram[a: a + ln, :]],
                outs=[dst_dram[8 * a: 8 * (a + ln), :]],
            )

    maxG = max(sum(n for _, _, n in gc) for gc in calls)
    maxG2 = max(sum(n for _, _, n in gc) for gc in calls2)

    def gather_group(gcalls, gi, rows_t, table, elem, stride, idx_sb,
                     bounds):
        """Issue per-part gather calls for group gi into rows_t."""
        gc = gcalls[gi]
        gbase = gc[0][1]  # first col of the group
        for p, c0, ncols in gc:
            if ncols == 0:
                continue
            tbl = table[bounds[p]: bounds[p + 1], :]
            o = c0 - gbase
            done = 0
            while done < ncols:
                n_ = min(IDXCAP // P, ncols - done)
                dg_raw(
                    rows_t[:, o + done: o + done + n_, :],
                    tbl,
                    idx_sb[:, (c0 + done) * 8: (c0 + done + n_) * 8],
                    n_ * P, elem, stride,
                )
                done += n_

    with tile.TileContext(nc) as tc:
        import contextlib

        ctx = contextlib.ExitStack()
        sb = ctx.enter_context(tc.tile_pool(name="sb", bufs=1))
        sb3 = ctx.enter_context(tc.tile_pool(name="sb3", bufs=3))

        # ---------- resident loads ----------
        src16_in_sb = sb.tile([P, C * 8], dt.int16)
        nc.sync.dma_start(src16_in_sb[:], src16_in[:])
        ones_bf_s = sb.tile([1, P], dt.bfloat16)
        nc.sync.dma_start(ones_bf_s[:], ones_bf[:])
        b1row_s = sb.tile([1, HID], dt.bfloat16)
        nc.sync.dma_start(b1row_s[:], b1row[:])
        b2row_s = sb.tile([1, OUTP], dt.bfloat16)
        nc.sync.dma_start(b2row_s[:], b2row[:])
        ident_s = sb.tile([P, P], dt.bfloat16)
        nc.sync.dma_start(ident_s[:], ident_in[:])
        m1_slab = sb.tile([P, NW, HID], dt.float8e4)
        nc.sync.dma_start(m1_slab[:], m1_in[:])
        m2_slab = sb.tile([P, NW, OUTP], dt.float8e4)
        nc.sync.dma_start(m2_slab[:], m2_in[:])

        W1_s = []
        for k in range(KT1):
            t = sb.tile([P, 2 * HID], dt.bfloat16, name=f"W1_s{k}")
            nc.sync.dma_start(t[:], W1[k * P: (k + 1) * P, :])
            W1_s.append(t)
        W2_s = []
        for k in range(KT2):
            t = sb.tile([P, 2 * OUTP], dt.bfloat16, name=f"W2_s{k}")
            nc.sync.dma_start(t[:], W2[k * P: (k + 1) * P, :])
            W2_s.append(t)

        root1_slab = sb.tile([P, NW, HID], dt.bfloat16)
        root2_slab = sb.tile([P, NW, OUTP], dt.bfloat16)
        iota4_s = sb.tile([P, 4, P], dt.bfloat16)
        nc.sync.dma_start(
            iota4_s[:], iota4_in[:].rearrange("p (a b) -> p a b", a=4))
        dl1_s = sb.tile([P, C], dt.bfloat16)
        nc.sync.dma_start(dl1_s[:], dl1_in[:])
        al1_s = sb.tile([P, C], dt.bfloat16)
        nc.sync.dma_start(al1_s[:], al1_in[:])
        dl2_s = sb.tile([P, C2], dt.bfloat16)
        nc.sync.dma_start(dl2_s[:], dl2_in[:])
        al2_s = sb.tile([P, C2], dt.bfloat16)
        nc.sync.dma_start(al2_s[:], al2_in[:])
        y_slab = sb.tile([P, NW, OUTP], dt.float32)
        htc_slab = sb.tile([P, NW, OUTP], dt.bfloat16)

        def build_oh(oh_t, dl_s, al_s, gbase, gcols, odt):
            for b in range(0, gcols, 4):
                nb = min(4, gcols - b)
                tmp = sb3.tile([P, 4, P], dt.bfloat16, tag="ohtmp", bufs=3)
                a_ = dl_s[:, gbase + b: gbase + b + nb]
                dlb = bass.AP(a_.tensor, a_.offset,
                              [list(a_.ap[0]), list(a_.ap[1]), [0, P]])
                nc.vector.tensor_tensor(
                    out=tmp[:, 0:nb, :], in0=dlb, in1=iota4_s[:, 0:nb, :],
                    op=A.is_equal,
                )
                b_ = al_s[:, gbase + b: gbase + b + nb]
                alb = bass.AP(b_.tensor, b_.offset,
                              [list(b_.ap[0]), list(b_.ap[1]), [0, P]])
                nc.vector.tensor_tensor(
                    out=oh_t[:, b: b + nb, :], in0=tmp[:, 0:nb, :], in1=alb,
                    op=A.mult,
                )
        hT_slab = []
        for k in range(KT2):
            t = sb.tile([P, PADN], dt.bfloat16, name=f"hT_slab{k}")
            hT_slab.append(t)


        # ---------- dense1 + chunked AllGather(xt hi|lo) ----------
        d1ctx = contextlib.ExitStack()
        psd = d1ctx.enter_context(tc.tile_pool(name="psd", bufs=1,
                                               space="PSUM"))
        slab_idx = 0
        for m in range(NW):
            ps = psd.tile([P, 2 * HID], dt.float32, space="PSUM", tag="d1",
                          bufs=2)
            xt_k = sb3.tile([P, KT1, P], dt.bfloat16, tag="xTt", bufs=3)
            nc.sync.dma_start(
                xt_k[:], xTw[m].rearrange("p (k n) -> p k n", k=KT1)
            )
            for k in range(KT1):
                nc.tensor.matmul(
                    ps[:], lhsT=xt_k[:, k, :], rhs=W1_s[k][:],
                    start=(k == 0), stop=False,
                )
            nc.tensor.matmul(
                ps[:, HID: 2 * HID],
                lhsT=ones_bf_s[:], rhs=b1row_s[:],
                start=False, stop=True,
            )
            xt_t = sb3.tile([P, 2 * HID], dt.float8e4, tag="xt_t")
            nc.scalar.copy(xt_t[:, 0:HID], ps[:, 0:HID])  # hi
            lo_bf = sb3.tile([P, HID], dt.bfloat16, tag="lo_bf", bufs=2)
            nc.vector.tensor_tensor(
                out=lo_bf[:], in0=ps[:, 0:HID], in1=xt_t[:, 0:HID],
                op=A.subtract,
            )
            nc.scalar.copy(xt_t[:, HID: 2 * HID], lo_bf[:])  # lo
            nc.sync.dma_start(xt_loc[m * P: (m + 1) * P, :], xt_t[:])
            nc.vector.tensor_copy(
                out=root1_slab[:, m, :], in_=ps[:, HID: 2 * HID],
            )
            if m + 1 == SLAB_WEND[slab_idx]:
                allgather(xt_loc, xt_full, slab_idx)
                slab_idx += 1

        # ---------- edge layer 1 (+ interleaved dense2) ----------
        d1ctx.close()
        e1ctx = contextlib.ExitStack()
        sb2 = e1ctx.enter_context(tc.tile_pool(name="sb2", bufs=2))
        sb2x = e1ctx.enter_context(tc.tile_pool(name="sb2x", bufs=1))
        # pre-zeroed expansion buffer for the padded ht table
        padt = sb2x.tile([P, 64, P], dt.bfloat16, name="padt")
        nc.vector.memset(padt[:], 0.0)
        psp = e1ctx.enter_context(tc.tile_pool(name="psp", bufs=1,
                                               space="PSUM"))
        slab_idx = 0
        for gi, ws in enumerate(groups):
            gbase = calls[gi][0][1]
            gcols = sum(n for _, _, n in calls[gi])
            rows = sb2.tile([P, maxG, 2 * HID], dt.float8e4, tag="rows1",
                            bufs=3)
            gather_group(calls, gi, rows, xt_full, 2 * HID, 2,
                         src16_in_sb, PART_BOUNDS)
            rowsv = rows[:].rearrange("p c (t h) -> p (c t) h", t=2)
            oh1_t = sb2.tile([P, maxG, P], dt.float8e4, tag="oh1", bufs=2)
            build_oh(oh1_t, dl1_s, al1_s, gbase, gcols, dt.float8e4)

            for w in ws:
                ps_b = psp.tile([P, HID], dt.float32, space="PSUM", tag="big",
                                bufs=2)
                first = True
                for p in range(NSLAB):
                    c0 = col[(p, w)] - gbase
                    for j in range(int(cw[p, w])):
                        cc = c0 + j
                        a_ = oh1_t[:, cc, :]
                        lhsT_b = bass.AP(
                            a_.tensor, a_.offset,
                            [list(a_.ap[0]), [0, 2], list(a_.ap[1])],
                        )
                        nc.tensor.matmul(
                            ps_b[:],
                            lhsT=lhsT_b,
                            rhs=rowsv[:, 2 * cc: 2 * cc + 2, :],
                            start=first, stop=False, perf_mode=DR,
                        )
                        first = False
                # + root1 (includes b1): identity matmul re-add
                nc.tensor.matmul(
                    ps_b[:], lhsT=ident_s[:], rhs=root1_slab[:, w, :],
                    start=False, stop=True,
                )
                t0 = sb3.tile([P, HID], dt.bfloat16, tag="t0", bufs=2)
                nc.vector.tensor_tensor(
                    out=t0[:], in0=ps_b[:], in1=m1_slab[:, w, :], op=A.mult
                )
                h_t = sb3.tile([P, HID], dt.bfloat16, tag="h_t", bufs=2)
                nc.scalar.activation(h_t[:], t0[:], F.Relu, scale=INV_KEEP)

                # dense2 for this window: hT via PE transpose, then matmuls
                tp = psp.tile([P, 2 * P], dt.bfloat16, space="PSUM", tag="tp",
                              bufs=1)
                for k in range(KT2):
                    nc.tensor.transpose(
                        out=tp[:, k * P: (k + 1) * P],
                        in_=h_t[:, k * P: (k + 1) * P],
                        identity=ident_s[:],
                    )
                    nc.scalar.copy(
                        hT_slab[k][:, w * P: (w + 1) * P],
                        tp[:, k * P: (k + 1) * P],
                    )
                psm = psp.tile([P, 2 * OUTP], dt.float32, space="PSUM",
                               tag="pm", bufs=2)
                for k in range(KT2):
                    nc.tensor.matmul(
                        psm[:],
                        lhsT=hT_slab[k][:, w * P: (w + 1) * P],
                        rhs=W2_s[k][:],
                        start=(k == 0), stop=False,
                    )
                nc.tensor.matmul(
                    psm[:, OUTP: 2 * OUTP],
                    lhsT=ones_bf_s[:], rhs=b2row_s[:],
                    start=False, stop=True,
                )
                nc.scalar.copy(htc_slab[:, w, :], psm[:, 0:OUTP])
                nc.vector.tensor_copy(
                    out=root2_slab[:, w, :], in_=psm[:, OUTP: 2 * OUTP],
                )

        # one compact AllGather of ht, then expand to the 256B-stride table
        nc.sync.dma_start(
            htc_loc[:], htc_slab[:].rearrange("p w o -> p (w o)"))
        if sim_mode:
            for cc in range(NCORES):
                nc.sync.dma_start(
                    htc_full[cc * PADN: (cc + 1) * PADN, :].rearrange(
                        "(p a) o -> p (a o)", p=P),
                    htc_loc[:],
                )
        else:
            nc.gpsimd.collective_compute(
                "AllGather", A.bypass, replica_groups=rg,
                ins=[htc_loc[:]],
                outs=[htc_full[0: 8 * PADN, :]],
            )
        for rc in range(0, 57344, 8192):
            cmp_t = sb2x.tile([P, 64, OUTP], dt.bfloat16, tag="cmp",
                              bufs=1)
            nc.sync.dma_start(
                cmp_t[:],
                htc_full[rc: rc + 8192, :].rearrange(
                    "(p c) o -> p c o", p=P),
            )
            nc.vector.tensor_copy(out=padt[:, :, 0:OUTP], in_=cmp_t[:])
            nc.sync.dma_start(
                htp[rc: rc + 8192, :].rearrange("(p c) o -> p (c o)", p=P),
                padt[:].rearrange("p c o -> p (c o)"),
            )

        e1ctx.close()

        # ---------- edge layer 2 ----------
        e2ctx = contextlib.ExitStack()
        sb2b = e2ctx.enter_context(tc.tile_pool(name="sb2b", bufs=2))
        psp2 = e2ctx.enter_context(tc.tile_pool(name="psp2", bufs=1,
                                                space="PSUM"))
        src16b_sb = sb2b.tile([P, C2 * 8], dt.int16, name="src16b")
        nc.sync.dma_start(src16b_sb[:], src16b_in[:])
        for gi, ws in enumerate(groups2):
            gbase = calls2[gi][0][1]
            gcols = sum(n for _, _, n in calls2[gi])
            rows2 = sb2b.tile([P, maxG2, OUTP], dt.bfloat16, tag="rows2",
                              bufs=4)
            gather_group(calls2, gi, rows2, htp, OUTP, 1, src16b_sb,
                         [0, 32768, N_NODES + 48])
            oh2_t = sb2b.tile([P, maxG2, P], dt.bfloat16, tag="oh2", bufs=3)
            build_oh(oh2_t, dl2_s, al2_s, gbase, gcols, dt.bfloat16)

            for w in ws:
                psg = psp2.tile([P, OUTP], dt.float32, space="PSUM",
                                tag="pg", bufs=2)
                first = True
                for p in range(2):
                    c0 = col2[(p, w)] - gbase
                    for j in range(int(cw2[p, w])):
                        cc = c0 + j
                        nc.tensor.matmul(
                            psg[:],
                            lhsT=oh2_t[:, cc, :],
                            rhs=rows2[:, cc, :],
                            start=first, stop=False,
                        )
                        first = False
                # + root2 (includes b2)
                nc.tensor.matmul(
                    psg[:], lhsT=ident_s[:], rhs=root2_slab[:, w, :],
                    start=False, stop=True,
                )
                t2 = sb3.tile([P, OUTP], dt.float32, tag="t2", bufs=2)
                nc.vector.tensor_tensor(
                    out=t2[:], in0=psg[:], in1=m2_slab[:, w, :], op=A.mult
                )
                nc.scalar.activation(
                    y_slab[:, w, :], t2[:], F.Relu, scale=INV_KEEP)
        nc.sync.dma_start(yL[:], y_slab[:].rearrange("p w o -> p (w o)"))
        e2ctx.close()
        ctx.close()

    nc.compile()
    return nc


def _stage_inputs(inputs, per_core_arrays):
    x = np.asarray(inputs["x"], np.float32)
    W1m = np.asarray(inputs["W1_msg"], np.float32)
    W1r = np.asarray(inputs["W1_root"], np.float32)
    b1 = np.asarray(inputs["b1"], np.float32)
    W2m = np.asarray(inputs["W2_msg"], np.float32)
    W2r = np.asarray(inputs["W2_root"], np.float32)
    b2 = np.asarray(inputs["b2"], np.float32)
    drop1 = np.asarray(inputs["drop1"], np.float32)
    drop2 = np.asarray(inputs["drop2"], np.float32)

    W1cat = np.concatenate([W1m, W1r], axis=1).astype(bf16)  # [768,512]
    W2cat = np.zeros((HID, 2 * OUTP), np.float32)
    W2cat[:, 0:OUT] = W2m
    W2cat[:, OUTP: OUTP + OUT] = W2r
    W2cat = W2cat.astype(bf16)
    b1row = b1.reshape(1, HID).astype(bf16)
    b2row = np.zeros((1, OUTP), np.float32)
    b2row[0, :OUT] = b2
    b2row = b2row.astype(bf16)
    ones_bf_a = np.ones((1, P), bf16)
    ident_a = np.eye(P, dtype=np.float32).astype(bf16)

    common = {
        "W1": W1cat,
        "W2": W2cat,
        "b1row": b1row,
        "b2row": b2row,
        "ones_bf": ones_bf_a,
        "ident": ident_a,
    }

    in_maps = []
    for c in range(NCORES):
        lo, hi = c * SHARD, (c + 1) * SHARD
        # xTw[w, p, k*128+n] = x[lo + w*128 + p ... wait: lhsT needs
        # xTw[w, p_feat? no: [w, p, 768] with partition p = feature-chunk row
        xp = np.ones((PADN, IN_DIM), np.float32)
        xp[:SHARD] = x[lo:hi]
        # lhsT for window w, chunk k: [feat 128, node 128]
        # xTw[w, p, k*128+n] = xp[w*128+n, k*128+p]
        xw = xp.reshape(NW, P, KT1, P)  # [w, n, k, pf]
        xTw = np.ascontiguousarray(xw.transpose(0, 3, 2, 1)).reshape(
            NW, P, IN_DIM
        )
        m1p = np.ones((PADN, HID), np.float32)
        m1p[:SHARD] = drop1[lo:hi] >= P_DROP
        m1w = np.ascontiguousarray(
            m1p.reshape(NW, P, HID).transpose(1, 0, 2)
        )
        m2p = np.zeros((PADN, OUTP), np.float32)
        m2p[:SHARD, :OUT] = drop2[lo:hi] >= P_DROP
        m2w = np.ascontiguousarray(
            m2p.reshape(NW, P, OUTP).transpose(1, 0, 2)
        )
        src16, dl1, al1, src16b, dl2, al2 = per_core_arrays[c]
        in_maps.append(
            {
                **common,
                "xTw": xTw.astype(bf16),
                "m1w": m1w.astype(f8),
                "m2w": m2w.astype(f8),
                "src16": src16,
                "src16b": src16b,
                "dl1": dl1,
                "al1": al1,
                "dl2": dl2,
                "al2": al2,
                "iota4": np.tile(np.arange(P, dtype=np.float32), 4
                                 ).reshape(1, 4 * P).repeat(P, 0
                                 ).astype(bf16),
            }
        )
    return in_maps


def _run(inputs, trace=False, trace_kwargs=None):
    from concourse import bass_utils

    et = np.asarray(inputs["edge_type"]).astype(np.int64)
    ed = np.asarray(inputs["edge_distance"]).astype(np.int64)
    a1 = _edge_alphas(
        et, ed, np.asarray(inputs["te1"], np.float32),
        np.asarray(inputs["de1"], np.float32),
        np.asarray(inputs["g1_w"], np.float32),
        np.asarray(inputs["g1_b"]).reshape(-1)[0],
    )
    a2 = _edge_alphas(
        et, ed, np.asarray(inputs["te2"], np.float32),
        np.asarray(inputs["de2"], np.float32),
        np.asarray(inputs["g2_w"], np.float32),
        np.asarray(inputs["g2_b"]).reshape(-1)[0],
    )
    meta, per_core_arrays = _prep_edges(inputs["edge_index"], a1, a2)
    nc = _build_program(meta)
    in_maps = _stage_inputs(inputs, per_core_arrays)
    res = bass_utils.run_bass_kernel_spmd(
        nc,
        in_maps,
        core_ids=list(range(NCORES)),
        trace=trace,
        **(trace_kwargs or {}),
    )
    parts = []
    for c in range(NCORES):
        yLa = np.asarray(res.results[c]["yL"]).reshape(P, NW, OUTP)
        yn = yLa.transpose(1, 0, 2).reshape(PADN, OUTP)
        parts.append(np.ascontiguousarray(yn[:SHARD, :OUT]))
    y = np.concatenate(parts, axis=0).astype(np.float32)
    return y, res


def kernel(**inputs) -> np.ndarray:
    y, _ = _run(inputs, trace=False)
    return y
